# revision 6
# baseline (speedup 1.0000x reference)
"""Trainium2 Bass kernel for the hex-board pattern one-hot encoder.

Reference semantics (see problem): boards (B, 11, 11) in {-1,0,1} ->
out (B, 27, 12, 12) f32 where out[b,p,i,j] = 1 iff the 3-tuple
(P[i,j], P[i,j+1], P[i+1,j]) of the border-padded 13x13 board equals
pattern p (patterns = product([-1,0,1], repeat=3)), with wildcard
corners at (0,0) [elem0], (0,11) [elem1], (11,0) [elem2].

Host prepads each board to the flat 169-elem 13x13 grid (borders are
constants). On device, per position g: idx = 9*P[g] + 3*P[g+1] +
P[g+13] + 13 in 0..26 via contiguous shifted views, then
out[p] = (idx == p) via 27 elementwise compares, plus tiny fix-ups
for the 3 wildcard corner columns.

Pure data parallel across 8 NeuronCores (batch sharding); memory-bound
on the ~510 MB f32 output write.

NB on sync-wait limits: instructions whose operands have >=2 free dims
use the S3D3 encoding which has room for only ONE embedded sync wait
("Too many sync wait commands" in walrus otherwise). All strided ops
here are placed so they need at most one cross-engine wait.
"""

import numpy as np

import concourse.bacc as bacc
import concourse.mybir as mybir
from concourse.mybir import AluOpType
from concourse.tile import TileContext

N_CORES = 8
BATCH = 32768
B_CORE = BATCH // N_CORES  # 4096
T = 4  # boards per partition per macrotile
NPART = 128
NMACRO = B_CORE // (NPART * T)  # 8
PADW = T * 169 + 14  # flat padded boards per partition + shift-read tail

F32 = mybir.dt.float32

# patterns touched by corner fixups (must be on VectorE, same engine as
# the fixup writes): {0,1,2,3,5,6,8} (corner C+A) u {18..20,24..26} (B+A)
DVE_PS = [0, 1, 2, 3, 5, 6, 8, 18, 19, 20, 24, 25, 26, 4, 7, 21]
POOL_PS = [9, 10, 11, 12, 13, 14, 15, 16, 17, 22, 23]
assert sorted(DVE_PS + POOL_PS) == list(range(27))


def build_nc(nmacro=NMACRO, debug=False):
    nc = bacc.Bacc("TRN2", target_bir_lowering=False, debug=debug)

    # board b_local = ((m*128 + r)*T + t); per-board input row is the
    # 169-elem host-padded 13x13 grid
    boards_h = nc.dram_tensor("boards", [nmacro, NPART, PADW], F32, kind="ExternalInput")
    out_h = nc.dram_tensor(
        "out", [nmacro, NPART, T * 27 * 144], F32, kind="ExternalOutput"
    )

    with TileContext(nc) as tc:
        with (
            tc.tile_pool(name="ppool", bufs=2) as ppool,
            tc.tile_pool(name="gpool", bufs=2) as gpool,
            tc.tile_pool(name="ipool", bufs=2) as ipool,
            tc.tile_pool(name="opool", bufs=2) as opool,
        ):
            for m in range(nmacro):
                # ---- load host-padded boards (contiguous) ----
                Pf = ppool.tile([NPART, PADW], F32, name="Pf")
                nc.scalar.dma_start(out=Pf, in_=boards_h[m])

                # ---- idx over the full flat grid (contiguous ops) ----
                # idxbig[g] = ((3*P[g] + P[g+1])*3 + 13) + P[g+13]
                NG = T * 169
                ib = gpool.tile([NPART, NG], F32, name="ib")
                nc.vector.tensor_scalar(ib, Pf[:, 0:NG], 3.0, None, AluOpType.mult)
                nc.vector.tensor_tensor(ib, ib, Pf[:, 1 : NG + 1], AluOpType.add)
                nc.vector.tensor_scalar(ib, ib, 3.0, 13.0, AluOpType.mult, AluOpType.add)
                nc.vector.tensor_tensor(ib, ib, Pf[:, 13 : NG + 13], AluOpType.add)

                # ---- compact the 12x12 subgrid per board slot ----
                idx = ipool.tile([NPART, T, 144], F32, name="idx")
                ibv = ib.rearrange("p (t a b) -> p t a b", a=13, b=13)
                for t in range(T):
                    nc.vector.tensor_copy(idx[:, t], ibv[:, t, 0:12, 0:12])

                idxf = idx.rearrange("p t f -> p (t f)")

                # ---- 27 one-hot compares ----
                out_t = opool.tile([NPART, T, 27, 144], F32, name="out_t")
                # claim the out_t slot's DMA WAR dep on gpsimd with a 1-dim
                # op (multi-wait capable); compare p=9 overwrites it below.
                nc.gpsimd.memset(out_t[:, :, 9, 0], 0.0)
                for p in POOL_PS:
                    nc.gpsimd.tensor_scalar(
                        out_t[:, :, p, :], idxf, float(p), None, AluOpType.is_equal
                    )
                for p in DVE_PS:
                    nc.vector.tensor_scalar(
                        out_t[:, :, p, :], idxf, float(p), None, AluOpType.is_equal
                    )

                # ---- wildcard corner fixups (all on VectorE) ----
                idxr = idx  # [NPART, T, 144]
                # corner (0,0) -> pos 0: idx=15; ones at p in {6,15,24}
                nc.vector.memset(out_t[:, :, 6, 0], 1.0)
                nc.vector.memset(out_t[:, :, 24, 0], 1.0)
                # corner (0,11) -> pos 11: idx = 22+c; ones at
                # p in {19+c, 22+c, 25+c}; middle band already right.
                for k in range(3):
                    for pb in (18 + k, 24 + k):
                        nc.vector.tensor_scalar(
                            out_t[:, :, pb, 11], idxr[:, :, 11], float(21 + k), None,
                            AluOpType.is_equal,
                        )
                # corner (11,0) -> pos 132: idx = 4+3d; ones at
                # p in {3d+3, 3d+4, 3d+5}; middle (s=1) already right.
                for mm in range(3):
                    for pb in (3 * mm, 3 * mm + 2):
                        nc.vector.tensor_scalar(
                            out_t[:, :, pb, 132], idxr[:, :, 132], float(3 * mm + 1),
                            None, AluOpType.is_equal,
                        )

                # ---- store ----
                nc.sync.dma_start(
                    out=out_h[m], in_=out_t.rearrange("p t q f -> p (t q f)")
                )

    nc.finalize()  # Bacc.compile(): reg alloc + sync-wait splitting
    return nc


def prep_core_input(boards_core):
    """(B_CORE, 11, 11) f32 -> padded flat [NMACRO, NPART, PADW]."""
    n = boards_core.shape[0]
    P = np.zeros((n, 13, 13), dtype=np.float32)
    P[:, 1:12, 1:12] = boards_core
    P[:, 0, 1:12] = 1.0
    P[:, 12, 1:12] = 1.0
    P[:, 1:12, 0] = -1.0
    P[:, 1:12, 12] = -1.0
    flat = P.reshape(n // T, T * 169)
    out = np.zeros((n // T, PADW), dtype=np.float32)
    out[:, : T * 169] = flat
    return out.reshape(n // (NPART * T), NPART, PADW)


def kernel(boards):
    from concourse.bass_utils import run_bass_kernel_spmd

    boards = np.ascontiguousarray(np.asarray(boards), dtype=np.float32)
    assert boards.shape == (BATCH, 11, 11)

    nc = build_nc()
    in_maps = [
        {"boards": prep_core_input(boards[c * B_CORE : (c + 1) * B_CORE])}
        for c in range(N_CORES)
    ]
    res = run_bass_kernel_spmd(nc, in_maps, core_ids=list(range(N_CORES)))
    out = np.empty((BATCH, 27, 12, 12), dtype=np.float32)
    for c in range(N_CORES):
        out[c * B_CORE : (c + 1) * B_CORE] = res.results[c]["out"].reshape(
            B_CORE, 27, 12, 12
        )
    return out


# revision 11
# speedup vs baseline: 4.0631x; 4.0631x over previous
"""Trainium2 Bass kernel for the hex-board pattern one-hot encoder.

Reference semantics (see problem): boards (B, 11, 11) in {-1,0,1} ->
out (B, 27, 12, 12) f32 where out[b,p,i,j] = 1 iff the 3-tuple
(P[i,j], P[i,j+1], P[i+1,j]) of the border-padded 13x13 board equals
pattern p (patterns = product([-1,0,1], repeat=3)), with wildcard
corners at (0,0) [elem0], (0,11) [elem1], (11,0) [elem2].

Host prepads each board to the flat 169-elem 13x13 grid (borders are
constants). On device, per position g: idx = 9*P[g] + 3*P[g+1] +
P[g+13] + 13 in 0..26 via contiguous shifted views, then
out[p] = (idx == p) via 27 elementwise compares, plus tiny fix-ups
for the 3 wildcard corner columns.

Pure data parallel across 8 NeuronCores (batch sharding); memory-bound
on the ~510 MB f32 output write.

NB on sync-wait limits: instructions whose operands have >=2 free dims
use the S3D3 encoding which has room for only ONE embedded sync wait
("Too many sync wait commands" in walrus otherwise). All strided ops
here are placed so they need at most one cross-engine wait.
"""

import numpy as np

import concourse.bacc as bacc
import concourse.mybir as mybir
from concourse.mybir import AluOpType
from concourse.tile import TileContext

N_CORES = 8
BATCH = 32768
B_CORE = BATCH // N_CORES  # 4096
T = 4  # boards per partition per macrotile
NPART = 128
NMACRO = B_CORE // (NPART * T)  # 8
PADW = T * 169 + 14  # flat padded boards per partition + shift-read tail

F32 = mybir.dt.float32

# patterns touched by corner fixups (must be on VectorE, same engine as
# the fixup writes): {0,1,2,3,5,6,8} (corner C+A) u {18..20,24..26} (B+A).
# GpSimd is NOT used for compares: its tensor_scalar measures ~9us/op and
# its SBUF-port lock stalls concurrent VectorE ops to the same speed.
# ScalarE computes (idx==p) as Relu(1-(idx-p)^2) in two activations.
ACT_PS = [9, 10, 11, 12, 13, 14, 15]
DVE_PS = [p for p in range(27) if p not in ACT_PS]


def build_nc(nmacro=NMACRO, debug=False):
    nc = bacc.Bacc("TRN2", target_bir_lowering=False, debug=debug)

    # board b_local = ((m*128 + r)*T + t); per-board input row is the
    # 169-elem host-padded 13x13 grid
    boards_h = nc.dram_tensor("boards", [nmacro, NPART, PADW], F32, kind="ExternalInput")
    out_h = nc.dram_tensor(
        "out", [nmacro, NPART, T * 27 * 144], F32, kind="ExternalOutput"
    )

    with TileContext(nc) as tc:
        with (
            tc.tile_pool(name="cpool", bufs=1) as cpool,
            tc.tile_pool(name="ppool", bufs=2) as ppool,
            tc.tile_pool(name="gpool", bufs=2) as gpool,
            tc.tile_pool(name="ipool", bufs=2) as ipool,
            tc.tile_pool(name="opool", bufs=2) as opool,
        ):
            # per-partition -p constants for the ScalarE Square bias
            negp = cpool.tile([NPART, 27], F32, name="negp")
            for p in ACT_PS:
                nc.vector.memset(negp[:, p : p + 1], float(-p))

            for m in range(nmacro):
                # ---- load host-padded boards (contiguous) ----
                Pf = ppool.tile([NPART, PADW], F32, name="Pf")
                nc.scalar.dma_start(out=Pf, in_=boards_h[m])

                # ---- idx over the full flat grid (contiguous ops) ----
                # idxbig[g] = ((3*P[g] + P[g+1])*3 + 13) + P[g+13]
                NG = T * 169
                ib = gpool.tile([NPART, NG], F32, name="ib")
                nc.vector.tensor_scalar(ib, Pf[:, 0:NG], 3.0, None, AluOpType.mult)
                nc.vector.tensor_tensor(ib, ib, Pf[:, 1 : NG + 1], AluOpType.add)
                nc.vector.tensor_scalar(ib, ib, 3.0, 13.0, AluOpType.mult, AluOpType.add)
                nc.vector.tensor_tensor(ib, ib, Pf[:, 13 : NG + 13], AluOpType.add)

                # ---- compact the 12x12 subgrid per board slot ----
                idx = ipool.tile([NPART, T, 144], F32, name="idx")
                ibv = ib.rearrange("p (t a b) -> p t a b", a=13, b=13)
                for t in range(T):
                    nc.vector.tensor_copy(idx[:, t], ibv[:, t, 0:12, 0:12])

                idxf = idx.rearrange("p t f -> p (t f)")

                # ---- 27 one-hot compares ----
                out_t = opool.tile([NPART, T, 27, 144], F32, name="out_t")
                # claim out_t's DMA WAR dep on ScalarE with a 1-free-dim op
                # (multi-wait capable); its own compare overwrites it below.
                c0 = ACT_PS[0]
                nc.scalar.mul(out_t[:, :, c0, 0], out_t[:, :, c0, 0], 0.0)
                for p in ACT_PS:
                    col = out_t[:, :, p, :]
                    nc.scalar.activation(
                        col, idxf, mybir.ActivationFunctionType.Square,
                        bias=negp[:, p : p + 1], scale=1.0,
                    )
                    nc.scalar.activation(
                        col, col, mybir.ActivationFunctionType.Relu,
                        bias=1.0, scale=-1.0,
                    )
                for p in DVE_PS:
                    nc.vector.tensor_scalar(
                        out_t[:, :, p, :], idxf, float(p), None, AluOpType.is_equal
                    )

                # ---- wildcard corner fixups (all on VectorE) ----
                idxr = idx  # [NPART, T, 144]
                # corner (0,0) -> pos 0: idx=15; ones at p in {6,15,24}
                nc.vector.memset(out_t[:, :, 6, 0], 1.0)
                nc.vector.memset(out_t[:, :, 24, 0], 1.0)
                # corner (0,11) -> pos 11: idx = 22+c; ones at
                # p in {19+c, 22+c, 25+c}; middle band already right.
                for k in range(3):
                    for pb in (18 + k, 24 + k):
                        nc.vector.tensor_scalar(
                            out_t[:, :, pb, 11], idxr[:, :, 11], float(21 + k), None,
                            AluOpType.is_equal,
                        )
                # corner (11,0) -> pos 132: idx = 4+3d; ones at
                # p in {3d+3, 3d+4, 3d+5}; middle (s=1) already right.
                for mm in range(3):
                    for pb in (3 * mm, 3 * mm + 2):
                        nc.vector.tensor_scalar(
                            out_t[:, :, pb, 132], idxr[:, :, 132], float(3 * mm + 1),
                            None, AluOpType.is_equal,
                        )

                # ---- store ----
                nc.sync.dma_start(
                    out=out_h[m], in_=out_t.rearrange("p t q f -> p (t q f)")
                )

    nc.finalize()  # Bacc.compile(): reg alloc + sync-wait splitting
    return nc


def prep_core_input(boards_core):
    """(B_CORE, 11, 11) f32 -> padded flat [NMACRO, NPART, PADW]."""
    n = boards_core.shape[0]
    P = np.zeros((n, 13, 13), dtype=np.float32)
    P[:, 1:12, 1:12] = boards_core
    P[:, 0, 1:12] = 1.0
    P[:, 12, 1:12] = 1.0
    P[:, 1:12, 0] = -1.0
    P[:, 1:12, 12] = -1.0
    flat = P.reshape(n // T, T * 169)
    out = np.zeros((n // T, PADW), dtype=np.float32)
    out[:, : T * 169] = flat
    return out.reshape(n // (NPART * T), NPART, PADW)


def kernel(boards):
    from concourse.bass_utils import run_bass_kernel_spmd

    boards = np.ascontiguousarray(np.asarray(boards), dtype=np.float32)
    assert boards.shape == (BATCH, 11, 11)

    nc = build_nc()
    in_maps = [
        {"boards": prep_core_input(boards[c * B_CORE : (c + 1) * B_CORE])}
        for c in range(N_CORES)
    ]
    res = run_bass_kernel_spmd(nc, in_maps, core_ids=list(range(N_CORES)))
    out = np.empty((BATCH, 27, 12, 12), dtype=np.float32)
    for c in range(N_CORES):
        out[c * B_CORE : (c + 1) * B_CORE] = res.results[c]["out"].reshape(
            B_CORE, 27, 12, 12
        )
    return out


# revision 13
# speedup vs baseline: 4.3709x; 1.0757x over previous
"""Trainium2 Bass kernel for the hex-board pattern one-hot encoder.

Reference semantics (see problem): boards (B, 11, 11) in {-1,0,1} ->
out (B, 27, 12, 12) f32 where out[b,p,i,j] = 1 iff the 3-tuple
(P[i,j], P[i,j+1], P[i+1,j]) of the border-padded 13x13 board equals
pattern p (patterns = product([-1,0,1], repeat=3)), with wildcard
corners at (0,0) [elem0], (0,11) [elem1], (11,0) [elem2].

Host prepads each board to the flat 169-elem 13x13 grid (borders are
constants). On device, per position g: idx = 9*P[g] + 3*P[g+1] +
P[g+13] + 13 in 0..26 via contiguous shifted views, then
out[p] = (idx == p) via 27 elementwise compares, plus tiny fix-ups
for the 3 wildcard corner columns.

Pure data parallel across 8 NeuronCores (batch sharding); memory-bound
on the ~510 MB f32 output write.

NB on sync-wait limits: instructions whose operands have >=2 free dims
use the S3D3 encoding which has room for only ONE embedded sync wait
("Too many sync wait commands" in walrus otherwise). All strided ops
here are placed so they need at most one cross-engine wait.
"""

import numpy as np

import concourse.bacc as bacc
import concourse.mybir as mybir
from concourse.mybir import AluOpType
from concourse.tile import TileContext

N_CORES = 8
BATCH = 32768
B_CORE = BATCH // N_CORES  # 4096
T = 4  # boards per partition per macrotile
NPART = 128
NMACRO = B_CORE // (NPART * T)  # 8
PADW = T * 169 + 14  # flat padded boards per partition + shift-read tail

F32 = mybir.dt.float32

# patterns touched by corner fixups (must be on VectorE, same engine as
# the fixup writes): {0,1,2,3,5,6,8} (corner C+A) u {18..20,24..26} (B+A).
# GpSimd is NOT used for compares: its tensor_scalar measures ~9us/op and
# its SBUF-port lock stalls concurrent VectorE ops to the same speed.
# ScalarE computes (idx==p) as Relu(1-(idx-p)^2) in two activations.
ACT_PS = [9, 10, 11, 12, 13, 14, 15]
DVE_PS = [p for p in range(27) if p not in ACT_PS]


def build_nc(nmacro=NMACRO, debug=False):
    nc = bacc.Bacc("TRN2", target_bir_lowering=False, debug=debug)

    # board b_local = ((m*128 + r)*T + t); per-board input row is the
    # 169-elem host-padded 13x13 grid
    boards_h = nc.dram_tensor("boards", [nmacro, NPART, PADW], F32, kind="ExternalInput")
    out_h = nc.dram_tensor(
        "out", [nmacro, NPART, T * 27 * 144], F32, kind="ExternalOutput"
    )

    with TileContext(nc) as tc:
        with (
            tc.tile_pool(name="cpool", bufs=1) as cpool,
            tc.tile_pool(name="ppool", bufs=2) as ppool,
            tc.tile_pool(name="gpool", bufs=2) as gpool,
            tc.tile_pool(name="ipool", bufs=2) as ipool,
            tc.tile_pool(name="opool", bufs=3) as opool,
        ):
            # per-partition -p constants for the ScalarE Square bias
            negp = cpool.tile([NPART, 27], F32, name="negp")
            for p in ACT_PS:
                nc.vector.memset(negp[:, p : p + 1], float(-p))

            for m in range(nmacro):
                # ---- load host-padded boards (contiguous) ----
                Pf = ppool.tile([NPART, PADW], F32, name="Pf")
                nc.scalar.dma_start(out=Pf, in_=boards_h[m])

                # ---- idx over the full flat grid (contiguous ops) ----
                # idxbig[g] = ((3*P[g] + P[g+1])*3 + 13) + P[g+13]
                NG = T * 169
                ib = gpool.tile([NPART, NG], F32, name="ib")
                nc.vector.tensor_scalar(ib, Pf[:, 0:NG], 3.0, None, AluOpType.mult)
                nc.vector.tensor_tensor(ib, ib, Pf[:, 1 : NG + 1], AluOpType.add)
                nc.vector.tensor_scalar(ib, ib, 3.0, 13.0, AluOpType.mult, AluOpType.add)
                nc.vector.tensor_tensor(ib, ib, Pf[:, 13 : NG + 13], AluOpType.add)

                # ---- compact the 12x12 subgrid per board slot ----
                idx = ipool.tile([NPART, T, 144], F32, name="idx")
                ibv = ib.rearrange("p (t a b) -> p t a b", a=13, b=13)
                for t in range(T):
                    nc.vector.tensor_copy(idx[:, t], ibv[:, t, 0:12, 0:12])

                idxf = idx.rearrange("p t f -> p (t f)")

                # ---- 27 one-hot compares, stored in 3 chunks of 9 so the
                # out-DMA starts as soon as the first third is ready ----
                out_t = opool.tile([NPART, T, 27, 144], F32, name="out_t")
                ohv = out_h[m].rearrange("p (t q f) -> p t q f", t=T, q=27, f=144)
                # claim out_t's DMA WAR dep on ScalarE with a 1-free-dim op
                # (multi-wait capable); its own compare overwrites it below.
                c0 = ACT_PS[0]
                nc.scalar.mul(out_t[:, :, c0, 0], out_t[:, :, c0, 0], 0.0)

                # chunk 0: p 0..8 (all DVE) + corner C fixups + corner A p6
                for p in range(9):
                    nc.vector.tensor_scalar(
                        out_t[:, :, p, :], idxf, float(p), None, AluOpType.is_equal
                    )
                # corner (11,0) -> pos 132: idx = 4+3d; ones at
                # p in {3d+3, 3d+4, 3d+5}; middle (s=1) already right.
                for mm in range(3):
                    for pb in (3 * mm, 3 * mm + 2):
                        nc.vector.tensor_scalar(
                            out_t[:, :, pb, 132], idx[:, :, 132], float(3 * mm + 1),
                            None, AluOpType.is_equal,
                        )
                # corner (0,0) -> pos 0: idx=15; ones at p in {6,15,24}
                nc.vector.memset(out_t[:, :, 6, 0], 1.0)
                nc.sync.dma_start(out=ohv[:, :, 0:9, :], in_=out_t[:, :, 0:9, :])

                # chunk 1: p 9..15 on ScalarE, 16..17 on DVE
                for p in ACT_PS:
                    col = out_t[:, :, p, :]
                    nc.scalar.activation(
                        col, idxf, mybir.ActivationFunctionType.Square,
                        bias=negp[:, p : p + 1], scale=1.0,
                    )
                    nc.scalar.activation(
                        col, col, mybir.ActivationFunctionType.Relu,
                        bias=1.0, scale=-1.0,
                    )
                for p in (16, 17):
                    nc.vector.tensor_scalar(
                        out_t[:, :, p, :], idxf, float(p), None, AluOpType.is_equal
                    )
                nc.sync.dma_start(out=ohv[:, :, 9:18, :], in_=out_t[:, :, 9:18, :])

                # chunk 2: p 18..26 (all DVE) + corner B fixups + corner A p24
                for p in range(18, 27):
                    nc.vector.tensor_scalar(
                        out_t[:, :, p, :], idxf, float(p), None, AluOpType.is_equal
                    )
                # corner (0,11) -> pos 11: idx = 22+c; ones at
                # p in {19+c, 22+c, 25+c}; middle band already right.
                for k in range(3):
                    for pb in (18 + k, 24 + k):
                        nc.vector.tensor_scalar(
                            out_t[:, :, pb, 11], idx[:, :, 11], float(21 + k), None,
                            AluOpType.is_equal,
                        )
                nc.vector.memset(out_t[:, :, 24, 0], 1.0)
                nc.sync.dma_start(out=ohv[:, :, 18:27, :], in_=out_t[:, :, 18:27, :])

    nc.finalize()  # Bacc.compile(): reg alloc + sync-wait splitting
    return nc


def prep_core_input(boards_core):
    """(B_CORE, 11, 11) f32 -> padded flat [NMACRO, NPART, PADW]."""
    n = boards_core.shape[0]
    P = np.zeros((n, 13, 13), dtype=np.float32)
    P[:, 1:12, 1:12] = boards_core
    P[:, 0, 1:12] = 1.0
    P[:, 12, 1:12] = 1.0
    P[:, 1:12, 0] = -1.0
    P[:, 1:12, 12] = -1.0
    flat = P.reshape(n // T, T * 169)
    out = np.zeros((n // T, PADW), dtype=np.float32)
    out[:, : T * 169] = flat
    return out.reshape(n // (NPART * T), NPART, PADW)


def kernel(boards):
    from concourse.bass_utils import run_bass_kernel_spmd

    boards = np.ascontiguousarray(np.asarray(boards), dtype=np.float32)
    assert boards.shape == (BATCH, 11, 11)

    nc = build_nc()
    in_maps = [
        {"boards": prep_core_input(boards[c * B_CORE : (c + 1) * B_CORE])}
        for c in range(N_CORES)
    ]
    res = run_bass_kernel_spmd(nc, in_maps, core_ids=list(range(N_CORES)))
    out = np.empty((BATCH, 27, 12, 12), dtype=np.float32)
    for c in range(N_CORES):
        out[c * B_CORE : (c + 1) * B_CORE] = res.results[c]["out"].reshape(
            B_CORE, 27, 12, 12
        )
    return out
